# revision 1
# baseline (speedup 1.0000x reference)
"""Multi-head attention + output projection on 8 Trainium2 NeuronCores.

Problem (hardcoded): x [3, 2, 4096, 512] fp32 (q/k/v stacked), proj_w [512, 512],
proj_b [512].  reference = softmax(q k^T / sqrt(64)) v, heads=8, then
out @ proj_w.T + proj_b.

Sharding: B*H = 16 (batch, head) pairs over 8 cores -> each core gets one
batch and one adjacent head PAIR (2 heads = 128 feature dims).  The output
projection is tensor-parallel along the contraction dim: each core computes
its partial y = attn_out_pair @ W[:, pair_dims].T; the host sums the 4
partials per batch and adds the bias.

Device kernel (per core).  All matmul operands are bf16 (fp32 streams the
PE at ~2 cycles/col; bf16 at 1 — and bf16 unlocks fast weight load), PSUM
accumulation stays fp32:
  scores^T[nk, nq] = kT_chunk.T @ qT      (two heads row-tiled in the PE
                                           array: K=64 each at rows 0-63 /
                                           64-127, concurrent)
  P^T: split across two engines per chunk —
    ScalarE chunks:  pt = exp(0.125 * scores^T) -> bf16   (native ACT exp)
    VectorE chunks:  u  = int16(round(A*scores^T + B)); the int16 bit
      pattern IS the bf16 encoding of exp(0.125*s) (Schraudolph trick,
      ~±3% sawtooth, zero-mean calibrated); PV reads u.bitcast(bf16).
      This offloads ~40% of the exp work from the otherwise-saturated
      ScalarE onto the idle DVE at 1 elem/lane/cycle.
  acc[d, nq]  += [V | 1].T @ P^T          (K=128; row 64 = softmax denom)
  y[nq, o]     = num.T @ W_h^T            (per head), then
  y            = y_h0 * rden_h0[nq] + y_h1 * rden_h1[nq]   (DVE)
"""

import numpy as np

C, B, N, D, H = 3, 2, 4096, 512, 8
HD = 64          # head dim
NCORES = 8
NQB = 512        # nq block width (PSUM bank)
NBLK = N // NQB  # 8 nq blocks
NCHUNK = N // 128  # 32 nk chunks of 128

# Schraudolph constants: bf16 bits of exp(0.125*s) ~ round(A*s + B).
# A = 2^7 * 0.125 / ln2; B = 127*2^7 - C with C=7.36 calibrated so the
# p-weighted mean relative error is zero (int16 convert-on-write rounds
# to nearest — verified on HW).
SCH_A = float(16.0 / np.log(2.0))
SCH_B = float(128.0 * 127.0 - 7.36)
# chunks whose exp runs on the DVE (rest on ScalarE); spread evenly.
# 14/32: after t1 moved to ScalarE, its per-block phase load (20 exps +
# 4 proj copies) exceeded the PE chunk phase — 14 rebalances both exp
# engines just under the PE.
DVE_CKS = frozenset({2, 4, 6, 8, 11, 13, 15, 17, 20, 22, 24, 26, 28, 30})

_compiled = None


def _build_nc():
    import concourse.bacc as bacc
    import concourse.tile as tile
    from concourse import mybir

    f32 = mybir.dt.float32
    bf16 = mybir.dt.bfloat16
    i16 = mybir.dt.int16
    Exp = mybir.ActivationFunctionType.Exp
    Copy = mybir.ActivationFunctionType.Copy
    mult = mybir.AluOpType.mult
    add = mybir.AluOpType.add

    nc = bacc.Bacc("TRN2", target_bir_lowering=False, debug=False, num_devices=1)

    qT = nc.dram_tensor("qT", [128, N], bf16, kind="ExternalInput").ap()
    kT = nc.dram_tensor("kT", [128, N], bf16, kind="ExternalInput").ap()
    vI = nc.dram_tensor("vI", [128, NCHUNK, 2, HD + 1], bf16, kind="ExternalInput").ap()
    wT = nc.dram_tensor("wT", [HD, 2, D], bf16, kind="ExternalInput").ap()
    y = nc.dram_tensor("y", [N, D], f32, kind="ExternalOutput").ap()

    with tile.TileContext(nc) as tc:
        with (
            tc.tile_pool(name="const", bufs=1) as const_pool,
            tc.tile_pool(name="pt", bufs=4) as pt_pool,
            tc.tile_pool(name="ep", bufs=3) as ep_pool,
            tc.tile_pool(name="ps_s", bufs=2, space="PSUM") as ps_s,
            tc.tile_pool(name="ps_a", bufs=1, space="PSUM") as ps_a,
            tc.tile_pool(name="ps_y", bufs=1, space="PSUM") as ps_y,
            tc.tile_pool(name="dram", bufs=2, space="DRAM") as dram_pool,
        ):
            # resident inputs
            qT_sb = const_pool.tile([128, N], bf16)
            kT_sb = const_pool.tile([128, N], bf16)
            vI_sb = const_pool.tile([128, NCHUNK, 2, HD + 1], bf16)
            # zero weights for the HAM-warming filler matmuls first so the
            # PE can start warming at t~0, then trigger the exp table load
            # while the input DMAs stream in
            zeros_sb = const_pool.tile([128, NQB], mybir.dt.bfloat16)
            nc.vector.memset(zeros_sb[:], 0.0)
            warm = ep_pool.tile([128, 2], f32, tag="warm")
            nc.vector.memset(warm[:], 0.0)
            nc.scalar.activation(warm[:, 1:2], warm[:, 0:1], Exp)
            nc.sync.dma_start(kT_sb[:, 0:128], kT[:, 0:128])
            nc.sync.dma_start(qT_sb[:, 0:NQB], qT[:, 0:NQB])
            nc.sync.dma_start(kT_sb[:, 128:512], kT[:, 128:512])
            nc.gpsimd.dma_start(vI_sb[:, 0:4], vI[:, 0:4])
            for ck4 in range(4, NCHUNK, 4):
                sl = slice(ck4 * 128, (ck4 + 4) * 128)
                nc.sync.dma_start(kT_sb[:, sl], kT[:, sl])
                nc.gpsimd.dma_start(vI_sb[:, ck4:ck4 + 4], vI[:, ck4:ck4 + 4])
            for b in range(1, NBLK):
                nc.gpsimd.dma_start(qT_sb[:, b * NQB:(b + 1) * NQB],
                                    qT[:, b * NQB:(b + 1) * NQB])
            wT_sb = const_pool.tile([HD, 2, D], bf16)
            nc.sync.dma_start(wT_sb[:], wT[:])
            pending_projs = []
            for blk in range(NBLK):
                q0 = blk * NQB
                a_h0 = ps_a.tile([HD + 1, NQB], f32, tag="a_h0")
                a_h1 = ps_a.tile([HD + 1, NQB], f32, tag="a_h1")
                if blk == 0:
                    for f in range(10):
                        nc.tensor.matmul(
                            (a_h0 if f % 2 == 0 else a_h1)[:],
                            lhsT=zeros_sb[:, 0:HD + 1], rhs=zeros_sb[:],
                            start=False, stop=False)
                # Software pipeline: PV for chunk t is emitted after the
                # scores matmuls of chunk t+1, so the PE works on PV(t)
                # while ScalarE/DVE run exp(t+1).
                pv_queue = []

                def emit_pv(args):
                    pt_ap, ck_ = args
                    first = ck_ == 0
                    last = ck_ == NCHUNK - 1
                    nc.tensor.matmul(
                        a_h0[:], lhsT=vI_sb[:, ck_, 0, :],
                        rhs=pt_ap[:, 0:NQB], start=first, stop=last)
                    nc.tensor.matmul(
                        a_h1[:], lhsT=vI_sb[:, ck_, 1, :],
                        rhs=pt_ap[:, NQB:2 * NQB], start=first, stop=last)

                for ck in range(NCHUNK):
                    # one [128, 1024] scores tile per chunk: h0 in bank 0,
                    # h1 in bank 1, the two matmuls run as a concurrent
                    # row-tiled pair (K=64 at array rows 0 / 64).
                    s_t = ps_s.tile([128, 2 * NQB], f32, tag="s_t")
                    nc.tensor.matmul(
                        s_t[:, 0:NQB],
                        lhsT=kT_sb[0:HD, ck * 128:(ck + 1) * 128],
                        rhs=qT_sb[0:HD, q0:q0 + NQB],
                        start=True, stop=True)
                    nc.tensor.matmul(
                        s_t[:, NQB:2 * NQB],
                        lhsT=kT_sb[HD:128, ck * 128:(ck + 1) * 128],
                        rhs=qT_sb[HD:128, q0:q0 + NQB],
                        start=True, stop=True)
                    if ck in DVE_CKS:
                        u = pt_pool.tile([128, 2 * NQB], i16, tag="ptu")
                        nc.vector.tensor_scalar(
                            u[:], s_t[:], SCH_A, SCH_B, mult, add)
                        pv_queue.append((u[:].bitcast(bf16), ck))
                    else:
                        pt = pt_pool.tile([128, 2 * NQB], bf16, tag="pt")
                        nc.scalar.activation(pt[:], s_t[:], Exp, scale=0.125)
                        pv_queue.append((pt[:], ck))
                    # hold PV back two iterations at the start of a block so
                    # the accumulator handoff (copies of the previous
                    # block's accumulators) never stalls the PE queue
                    if ck >= 2 and len(pv_queue) > 2:
                        emit_pv(pv_queue.pop(0))
                    if ck >= 2 and len(pv_queue) > 2:
                        emit_pv(pv_queue.pop(0))
                    # previous block's proj/normalize groups, spread out so
                    # the in-order PE queue never stalls on the DVE chain
                    if ck in (4, 9, 14, 19) and pending_projs:
                        pending_projs.pop(0)()
                for a in pv_queue:
                    emit_pv(a)

                # stage accumulators to SBUF right away: this is the only
                # thing the next block's PV accumulation waits on.  bf16
                # copies feed the proj matmuls; the denominator row is
                # copied separately in fp32 to keep 1/den exact.
                st0 = ep_pool.tile([HD + 1, NQB], bf16, tag="st0")
                nc.vector.tensor_copy(st0[0:HD, :], a_h0[0:HD, :])
                st1 = ep_pool.tile([HD + 1, NQB], bf16, tag="st1")
                nc.vector.tensor_copy(st1[0:HD, :], a_h1[0:HD, :])
                dden = ep_pool.tile([HD + 1, 2 * NQB], f32, tag="dden")
                nc.vector.tensor_copy(dden[HD:HD + 1, 0:NQB], a_h0[HD:HD + 1, :])
                nc.vector.tensor_copy(dden[HD:HD + 1, NQB:2 * NQB],
                                      a_h1[HD:HD + 1, :])
                dtmp = dram_pool.tile([2, NQB], f32, tag="dtmp")
                nc.sync.dma_start(dtmp[0:1, :], dden[HD:HD + 1, 0:NQB])
                nc.sync.dma_start(dtmp[1:2, :], dden[HD:HD + 1, NQB:2 * NQB])
                dT = ep_pool.tile([128, 4, 2], f32, tag="dT")
                for h in range(2):
                    nc.sync.dma_start(
                        dT[:, :, h], dtmp[h].rearrange("(c p) -> p c", p=128))
                denT = ep_pool.tile([128, 4, 2], f32, tag="denT")
                nc.vector.reciprocal(denT[:], dT[:])

                def make_proj(cc, st0=st0, st1=st1, denT=denT, q0=q0, alt=False):
                    def emit_proj():
                        n0 = q0 + cc * 128
                        t0_, t1_ = ("a_h0", "a_h1") if alt else ("y0", "y1")
                        pool0 = ps_a if alt else ps_y
                        y0 = pool0.tile([128, D], f32, tag=t0_, name=f"yy0_{q0}_{cc}")
                        nc.tensor.matmul(
                            y0[:], lhsT=st0[0:HD, cc * 128:(cc + 1) * 128],
                            rhs=wT_sb[:, 0, :], start=True, stop=True)
                        y1 = pool0.tile([128, D], f32, tag=t1_, name=f"yy1_{q0}_{cc}")
                        nc.tensor.matmul(
                            y1[:], lhsT=st1[0:HD, cc * 128:(cc + 1) * 128],
                            rhs=wT_sb[:, 1, :], start=True, stop=True)
                        t1 = ep_pool.tile([128, D], f32, tag="t1")
                        # t1 on the ScalarE (Copy with per-partition scale):
                        # pipelines the normalize across PE -> ScalarE ->
                        # DVE -> DMA instead of two serialized DVE ops —
                        # this is the tail's critical path
                        nc.scalar.activation(t1[:], y1[:], Copy,
                                             scale=denT[:, cc, 1:2])
                        y_sb = ep_pool.tile([128, D], f32, tag="y_sb")
                        nc.vector.scalar_tensor_tensor(
                            y_sb[:], y0[:], denT[:, cc, 0:1], t1[:], op0=mult, op1=add)
                        # y writeback on the gpsimd DMA queue: keeps the
                        # sync queue free for the den round-trip, whose
                        # latency lands on the tail critical path
                        nc.gpsimd.dma_start(y[n0:n0 + 128, :], y_sb[:])
                    return emit_proj

                if blk < NBLK - 1:
                    pending_projs = [make_proj(cc) for cc in range(4)]
                else:
                    pending_projs = [make_proj(cc, alt=(cc % 2 == 1))
                                     for cc in range(4)]
            for p in pending_projs:
                p()

    nc.compile()
    return nc


def _get_compiled():
    global _compiled
    if _compiled is None:
        _compiled = _build_nc()
    return _compiled


def _prep_core_inputs(x, proj_w):
    """Host-side shard + layout per core: core c -> batch c//4, head pair c%4."""
    import ml_dtypes
    bf16 = ml_dtypes.bfloat16

    ins = []
    for c in range(NCORES):
        b, hp = c // 4, c % 4
        sl = slice(128 * hp, 128 * hp + 128)
        qT = np.ascontiguousarray(x[0, b, :, sl].T).astype(bf16)
        kT = np.ascontiguousarray(x[1, b, :, sl].T).astype(bf16)
        v = x[2, b, :, sl]                       # [N, 128]
        vI = np.ones((128, NCHUNK, 2, HD + 1), np.float32)
        vr = v.reshape(NCHUNK, 128, 2, HD)        # [chunk, p, head, m]
        vI[:, :, :, :HD] = vr.transpose(1, 0, 2, 3)
        wT = np.ascontiguousarray(
            proj_w[:, sl].T.reshape(2, HD, D).transpose(1, 0, 2))  # [HD, 2, D]
        ins.append({"qT": qT, "kT": kT, "vI": vI.astype(bf16),
                    "wT": wT.astype(bf16)})
    return ins


def kernel(x, proj_w, proj_b):
    from concourse.bass_utils import run_bass_kernel_spmd

    x = np.asarray(x, dtype=np.float32)
    proj_w = np.asarray(proj_w, dtype=np.float32)
    proj_b = np.asarray(proj_b, dtype=np.float32)

    nc = _get_compiled()
    in_maps = _prep_core_inputs(x, proj_w)
    res = run_bass_kernel_spmd(nc, in_maps, core_ids=list(range(NCORES)))

    out = np.zeros((B, N, D), np.float32)
    for c in range(NCORES):
        out[c // 4] += res.results[c]["y"]
    out += proj_b
    return out

